# revision 1
# baseline (speedup 1.0000x reference)
"""Distributed Trainium2 kernel for the ACloss loss function.

Shards the batch dim (16 -> 2 images/core) across 8 NeuronCores. Each core:
  - computes sum((output-target)^2) over its shard  (l2 partial)
  - finds the row-major-first argmax of each of its 76 heatmaps
    (2 images x 19 landmarks x {output,target})
  - builds the 19x19 angle / distance matrices per image and reduces
    them to partial angle/dist sums
The host sums the 8 partial results and applies the final scalar math.

Engine split per core: DVE does the per-partition max reduces, GpSimd the
l2 subtracts (freeing the DVE), ActE the squares+transcendentals, PE the
tiny transposes and outer-product matmuls. Scheduling floors
(tile_wait_until) keep the small argmax/angle chain out of the DVE/GpSimd
streams until the streaming phase is done, avoiding head-of-line stalls;
the tail itself is batched over all 76 heatmaps (acos via the A&S 4.4.45
polynomial, ActE Sqrt table preloaded by a dummy op during the gathers).
"""

import os
import numpy as np

B, L, H, W = 16, 19, 256, 256
NCORES = 8
B_LOC = B // NCORES            # 2 images per core
G = 2 * B_LOC                  # 4 groups: g0=out b0, g1=out b1, g2=tgt b0, g3=tgt b1
NH = G * L                     # 76 heatmaps per core
HH = NH // 2                   # 38 heatmaps per source tensor
P = 128                        # partitions per heatmap tile
F = (H * W) // P               # 512 free elems per partition

_CACHE = {}
LAST_RESULTS = None

# DMA / reduce chunks (landmark ranges): small first chunk for an early
# compute start, tiny last chunk to shorten the post-DMA reduce tail.
CH = [(0, 3), (3, 6), (9, 6), (15, 3), (18, 1)]
# l2 subtract chunks, nested inside CH boundaries; keyed by covering wave
L2CH = {0: [(0, 3)], 1: [(3, 3), (6, 3)], 2: [(9, 3), (12, 3)],
        3: [(15, 3)], 4: [(18, 1)]}
N_L2 = sum(len(v) for v in L2CH.values())


def _build():
    from contextlib import ExitStack

    import concourse.bass as bass
    import concourse.tile as tile
    from concourse import bacc, mybir

    fp32 = mybir.dt.float32
    i32 = mybir.dt.int32
    u32 = mybir.dt.uint32
    Alu = mybir.AluOpType
    Act = mybir.ActivationFunctionType
    AX = mybir.AxisListType

    nc = bacc.Bacc("TRN2", target_bir_lowering=False, debug=False,
                   num_devices=NCORES)

    data_p = nc.declare_dram_parameter("data", [2, B_LOC, L, H, W], fp32,
                                       isOutput=False)
    pri_p = nc.declare_dram_parameter("pri", [NH, P], fp32, isOutput=False)
    r0c_p = nc.declare_dram_parameter("r0c", [NH, 1], fp32, isOutput=False)
    ones_p = nc.declare_dram_parameter("onesv", [P, 1], fp32, isOutput=False)
    ident_p = nc.declare_dram_parameter("ident", [P, P], fp32, isOutput=False)
    res_p = nc.declare_dram_parameter("res", [8], fp32, isOutput=True)

    # [s, b, 128, l, 512] views: partition p holds rows {2p, 2p+1}
    dv = data_p.ap().rearrange("s b l (p h2) w -> s b p l (h2 w)", p=P, h2=2)
    out_v, tgt_v = dv[0], dv[1]
    # one flat row view over BOTH sources for a single indirect gather
    all_flat = data_p.ap().rearrange("s b l (p h2) w -> (s b l p) (h2 w)",
                                     p=P, h2=2)

    with tile.TileContext(nc) as tc, ExitStack() as ctx:
        data = ctx.enter_context(tc.tile_pool(name="data", bufs=1))
        small = ctx.enter_context(tc.tile_pool(name="small", bufs=1))
        half = ctx.enter_context(tc.tile_pool(name="halfp", bufs=2))
        dpool = ctx.enter_context(tc.tile_pool(name="dpool", bufs=2))
        psum = ctx.enter_context(tc.tile_pool(name="psum", bufs=1, space="PSUM"))

        # constants
        pri = small.tile([NH, P], fp32, tag="pri")
        r0c = small.tile([NH, 1], fp32, tag="r0c")
        ones = small.tile([P, 1], fp32, tag="ones")
        ident = small.tile([P, P], fp32, tag="ident")
        nc.sync.dma_start(out=pri[:], in_=pri_p[:])
        nc.sync.dma_start(out=r0c[:], in_=r0c_p[:])
        nc.sync.dma_start(out=ones[:], in_=ones_p[:])
        nc.sync.dma_start(out=ident[:], in_=ident_p[:])

        grp = [data.tile([P, L, F], fp32, tag=f"grp{g}", name=f"grp{g}")
               for g in range(G)]
        colmax = small.tile([P, NH], fp32, tag="colmax")
        l2cols = small.tile([P, 2 * N_L2], fp32, tag="l2cols")

        def gsrc(g):
            return (out_v, g) if g < 2 else (tgt_v, g - 2)

        # ---- streaming phase: DMA waves + max reduces + l2 chain ----
        li = 0
        for ci, (lo, nl) in enumerate(CH):
            for g in (0, 2, 1, 3):
                v, b = gsrc(g)
                nc.sync.dma_start(out=grp[g][:, lo:lo + nl, :],
                                  in_=v[b, :, lo:lo + nl, :])
            for g in (0, 2, 1, 3):
                nc.vector.tensor_reduce(
                    out=colmax[:, g * L + lo: g * L + lo + nl],
                    in_=grp[g][:, lo:lo + nl, :],
                    axis=AX.X, op=Alu.max)
            # waves 0-1 subtract on gpsimd; waves 2 on DVE inline; the
            # last waves are deferred (floor 1.5) so they fill the DVE
            # idle gap while the row gathers are in flight
            floor = 1.5 if ci >= 3 else 0
            with tc.tile_wait_until(floor, enable=ci >= 3):
                for (l2lo, l2nl) in L2CH[ci]:
                    for b in range(B_LOC):
                        d = dpool.tile([P, 3, F], fp32, tag="d",
                                       name=f"d{b}_{l2lo}")
                        d2 = dpool.tile([P, 3, F], fp32, tag="d2",
                                        name=f"d2{b}_{l2lo}")
                        sub_eng = nc.gpsimd if ci < 2 else nc.vector
                        sub_eng.tensor_tensor(
                            out=d[:, 0:l2nl, :],
                            in0=grp[b][:, l2lo:l2lo + l2nl, :],
                            in1=grp[2 + b][:, l2lo:l2lo + l2nl, :],
                            op=Alu.subtract)
                        nc.scalar.activation(
                            out=d2[:, 0:l2nl, :], in_=d[:, 0:l2nl, :],
                            func=Act.Square, accum_out=l2cols[:, li:li + 1])
                        li += 1

        # ---- argmax tail (batched over all 76 heatmaps) ----
        with tc.tile_wait_until(1):
            colmaxT = psum.tile([NH, P], fp32, tag="colmaxT", space="PSUM")
            nc.tensor.transpose(out=colmaxT[:], in_=colmax[:],
                                identity=ident[:])
            gmax = small.tile([NH, 1], fp32, tag="gmax")
            nc.vector.tensor_reduce(out=gmax[:], in_=colmaxT[:], axis=AX.X,
                                    op=Alu.max)
            # tmp[h,p] = (colmax_p==gmax_h)*(128-p); max -> 128 - wp_first
            tmp = small.tile([NH, P], fp32, tag="tmpw")
            nc.vector.scalar_tensor_tensor(out=tmp[:], in0=colmaxT[:],
                                           scalar=gmax[:, 0:1], in1=pri[:],
                                           op0=Alu.is_ge, op1=Alu.mult)
            wsel = small.tile([NH, 1], fp32, tag="wsel")
            nc.vector.tensor_reduce(out=wsel[:], in_=tmp[:], axis=AX.X,
                                    op=Alu.max)
            # dram row = (b*19+l)*128 + wp = r0c - wsel  (r0c bakes +128)
            offs_i = small.tile([NH, 1], i32, tag="offs_i")
            nc.vector.scalar_tensor_tensor(out=offs_i[:], in0=wsel[:],
                                           scalar=-1.0, in1=r0c[:],
                                           op0=Alu.mult, op1=Alu.add)
            # single gather over the concatenated sources (r0c bakes the
            # per-source row base, so one desc-gen fetches all 76 rows)
            rows = small.tile([NH, F], fp32, tag="rows")
            nc.gpsimd.indirect_dma_start(
                out=rows[:], out_offset=None, in_=all_flat[:],
                in_offset=bass.IndirectOffsetOnAxis(ap=offs_i[:, 0:1],
                                                    axis=0))
            # dummy op pulls the ActE Sqrt table load off the critical chain
            dummy = small.tile([1, 8], fp32, tag="dummy")
            nc.vector.memset(dummy[:], 0.25)
            nc.scalar.activation(out=dummy[:], in_=dummy[:], func=Act.Sqrt)

        with tc.tile_wait_until(2):
            max8 = small.tile([NH, 8], fp32, tag="max8")
            nc.vector.max(out=max8[:], in_=rows[:])
            idx8 = small.tile([NH, 8], u32, tag="idx8")
            nc.vector.max_index(out=idx8[:], in_max=max8[:], in_values=rows[:])
            widx = small.tile([NH, 1], fp32, tag="widx")
            nc.vector.tensor_copy(out=widx[:], in_=idx8[:, 0:1])

            # coords: y = 2*wp + (widx>=256), x = widx - 256*(widx>=256)
            # v = coords - 128; wp = 128 - wsel
            thi = small.tile([NH, 1], fp32, tag="thi")
            nc.vector.tensor_single_scalar(out=thi[:], in_=widx[:],
                                           scalar=256.0, op=Alu.is_ge)
            vc = small.tile([NH, 2], fp32, tag="vc")
            vyt = small.tile([NH, 1], fp32, tag="vyt")
            nc.vector.scalar_tensor_tensor(out=vyt[:], in0=wsel[:],
                                           scalar=-2.0, in1=thi[:],
                                           op0=Alu.mult, op1=Alu.add)
            nc.vector.tensor_single_scalar(out=vc[:, 0:1], in_=vyt[:],
                                           scalar=128.0, op=Alu.add)
            vxt = small.tile([NH, 1], fp32, tag="vxt")
            nc.vector.scalar_tensor_tensor(out=vxt[:], in0=thi[:],
                                           scalar=-256.0, in1=widx[:],
                                           op0=Alu.mult, op1=Alu.add)
            nc.vector.tensor_single_scalar(out=vc[:, 1:2], in_=vxt[:],
                                           scalar=-128.0, op=Alu.add)

            # nsq in column layout, then PE transposes to row layout
            vsq = small.tile([NH, 2], fp32, tag="vsq")
            nc.vector.tensor_tensor(out=vsq[:], in0=vc[:], in1=vc[:],
                                    op=Alu.mult)
            nsqc = small.tile([NH, 1], fp32, tag="nsqc")
            nc.vector.tensor_reduce(out=nsqc[:], in_=vsq[:], axis=AX.X,
                                    op=Alu.add)
            v2p = psum.tile([2, NH], fp32, tag="v2p", space="PSUM")
            nc.tensor.transpose(out=v2p[:], in_=vc[:],
                                identity=ident[0:NH, 0:NH])
            v2 = small.tile([2, NH], fp32, tag="v2")
            nc.scalar.copy(out=v2[:], in_=v2p[:])
            nsqp = psum.tile([1, NH], fp32, tag="nsqp", space="PSUM")
            nc.tensor.transpose(out=nsqp[:], in_=nsqc[:],
                                identity=ident[0:NH, 0:NH])
            nsq = small.tile([1, NH], fp32, tag="nsq")
            nc.scalar.copy(out=nsq[:], in_=nsqp[:])

            # guarded 1/norm and nonzero mask (all on partition 0)
            nrm = small.tile([1, NH], fp32, tag="nrm")
            nc.scalar.activation(out=nrm[:], in_=nsq[:], func=Act.Sqrt)
            zed = small.tile([1, NH], fp32, tag="zed")
            nc.vector.tensor_single_scalar(out=zed[:], in_=nsq[:], scalar=0.0,
                                           op=Alu.is_le)
            nzm = small.tile([1, NH], fp32, tag="nzm")
            nc.vector.tensor_scalar(out=nzm[:], in0=zed[:], scalar1=-1.0,
                                    scalar2=1.0, op0=Alu.mult, op1=Alu.add)
            nsafe = small.tile([1, NH], fp32, tag="nsafe")
            nc.vector.tensor_tensor(out=nsafe[:], in0=nrm[:], in1=zed[:],
                                    op=Alu.add)
            rec = small.tile([1, NH], fp32, tag="rec")
            nc.vector.reciprocal(out=rec[:], in_=nsafe[:])
            rr = small.tile([1, NH], fp32, tag="rr")
            nc.vector.tensor_tensor(out=rr[:], in0=rec[:], in1=nzm[:],
                                    op=Alu.mult)

            onesrow = small.tile([1, NH], fp32, tag="onesrow")
            nc.vector.memset(onesrow[:], 1.0)

            dots = psum.tile([L, NH], fp32, tag="dots", space="PSUM")
            rrP = psum.tile([L, NH], fp32, tag="rrP", space="PSUM")
            osP = psum.tile([L, NH], fp32, tag="osP", space="PSUM")
            for g in range(G):
                sl = slice(g * L, (g + 1) * L)
                nc.tensor.matmul(out=dots[:, sl], lhsT=v2[:, sl],
                                 rhs=v2[:, sl], start=True, stop=True)
                nc.tensor.matmul(out=rrP[:, sl], lhsT=rr[0:1, sl],
                                 rhs=rr[0:1, sl], start=True, stop=True)
                nc.tensor.matmul(out=osP[:, sl], lhsT=nsq[0:1, sl],
                                 rhs=onesrow[0:1, sl], start=True, stop=False)
                nc.tensor.matmul(out=osP[:, sl], lhsT=onesrow[0:1, sl],
                                 rhs=nsq[0:1, sl], start=False, stop=True)

        with tc.tile_wait_until(3):
            # angle via the A&S 4.4.45 polynomial:
            #   acos(x) = sqrt(1-x)*(a0 + a1 x + a2 x^2 + a3 x^3), x in [0,1]
            #   acos(x<0) = pi - acos(-x);  abs err <= 5e-5
            # nz mask = (rrP > 0), true iff both landmarks are nonzero
            dotsS = small.tile([L, NH], fp32, tag="dotsS")
            nc.scalar.copy(out=dotsS[:], in_=dots[:])
            msk = small.tile([L, NH], fp32, tag="msk")
            nc.vector.tensor_single_scalar(out=msk[:], in_=rrP[:], scalar=0.0,
                                           op=Alu.is_gt)
            cosm = small.tile([L, NH], fp32, tag="cosm")
            nc.vector.tensor_tensor(out=cosm[:], in0=dotsS[:], in1=rrP[:],
                                    op=Alu.mult)
            mng = small.tile([L, NH], fp32, tag="mng")
            nc.vector.tensor_single_scalar(out=mng[:], in_=cosm[:], scalar=0.0,
                                           op=Alu.is_lt)
            flp = small.tile([L, NH], fp32, tag="flp")
            nc.vector.tensor_scalar(out=flp[:], in0=mng[:], scalar1=-2.0,
                                    scalar2=1.0, op0=Alu.mult, op1=Alu.add)
            ax = small.tile([L, NH], fp32, tag="ax")
            nc.vector.tensor_tensor(out=ax[:], in0=cosm[:], in1=flp[:],
                                    op=Alu.mult)
            nc.vector.tensor_single_scalar(out=ax[:], in_=ax[:], scalar=1.0,
                                           op=Alu.min)
            A0, A1, A2, A3 = 1.5707288, -0.2121144, 0.0742610, -0.0187293
            h1 = small.tile([L, NH], fp32, tag="h1")
            nc.vector.tensor_scalar(out=h1[:], in0=ax[:], scalar1=A3,
                                    scalar2=A2, op0=Alu.mult, op1=Alu.add)
            h2 = small.tile([L, NH], fp32, tag="h2")
            nc.vector.tensor_tensor(out=h2[:], in0=h1[:], in1=ax[:],
                                    op=Alu.mult)
            nc.vector.tensor_single_scalar(out=h2[:], in_=h2[:], scalar=A1,
                                           op=Alu.add)
            h3 = small.tile([L, NH], fp32, tag="h3")
            nc.vector.tensor_tensor(out=h3[:], in0=h2[:], in1=ax[:],
                                    op=Alu.mult)
            nc.vector.tensor_single_scalar(out=h3[:], in_=h3[:], scalar=A0,
                                           op=Alu.add)
            qq = small.tile([L, NH], fp32, tag="qq")
            nc.vector.tensor_scalar(out=qq[:], in0=ax[:], scalar1=-1.0,
                                    scalar2=1.0, op0=Alu.mult, op1=Alu.add)
            sq = small.tile([L, NH], fp32, tag="sq")
            nc.scalar.activation(out=sq[:], in_=qq[:], func=Act.Sqrt)
            acp = small.tile([L, NH], fp32, tag="acp")
            nc.vector.tensor_tensor(out=acp[:], in0=sq[:], in1=h3[:],
                                    op=Alu.mult)
            ac2 = small.tile([L, NH], fp32, tag="ac2")
            nc.vector.tensor_tensor(out=ac2[:], in0=acp[:], in1=flp[:],
                                    op=Alu.mult)
            ac3 = small.tile([L, NH], fp32, tag="ac3")
            nc.vector.scalar_tensor_tensor(out=ac3[:], in0=mng[:],
                                           scalar=float(np.pi), in1=ac2[:],
                                           op0=Alu.mult, op1=Alu.add)
            ang = small.tile([L, NH], fp32, tag="ang")
            nc.vector.tensor_tensor(out=ang[:], in0=ac3[:], in1=msk[:],
                                    op=Alu.mult)

            # dist = sqrt(max(osP - 2*dots, 0))
            d2m = small.tile([L, NH], fp32, tag="d2m")
            nc.vector.scalar_tensor_tensor(out=d2m[:], in0=dotsS[:],
                                           scalar=-2.0, in1=osP[:],
                                           op0=Alu.mult, op1=Alu.add)
            nc.vector.tensor_single_scalar(out=d2m[:], in_=d2m[:], scalar=0.0,
                                           op=Alu.max)
            dist = small.tile([L, NH], fp32, tag="dist")
            nc.scalar.activation(out=dist[:], in_=d2m[:], func=Act.Sqrt)

        with tc.tile_wait_until(4):
            # per-image |out - tgt| sums for angle and dist
            # heatmap col layout: [out b0 | out b1 | tgt b0 | tgt b1]
            sums19 = small.tile([L, 4], fp32, tag="sums19")
            for i, mat in enumerate((ang, dist)):
                dtmp = small.tile([L, 2 * L], fp32, tag="dtmp",
                                  name=f"dtmp{i}")
                nc.vector.tensor_tensor(out=dtmp[:], in0=mat[:, 0:2 * L],
                                        in1=mat[:, 2 * L:NH],
                                        op=Alu.subtract)
                nc.vector.tensor_reduce(
                    out=sums19[:, 2 * i:2 * i + 2],
                    in_=dtmp[:].rearrange("p (i l) -> p i l", l=L),
                    axis=AX.X, op=Alu.add, apply_absolute_value=True)

            # final partition reductions via one PE ones-matmul
            combo = small.tile([P, 5], fp32, tag="combo")
            nc.vector.memset(combo[:], 0.0)
            nc.vector.tensor_reduce(out=combo[:, 0:1], in_=l2cols[:],
                                    axis=AX.X, op=Alu.add)
            nc.vector.tensor_copy(out=combo[0:L, 1:5], in_=sums19[:])
            finP = psum.tile([5, 1], fp32, tag="finP", space="PSUM")
            nc.tensor.matmul(out=finP[:], lhsT=combo[:], rhs=ones[:],
                             start=True, stop=True)
            finsb = small.tile([5, 1], fp32, tag="finsb")
            nc.scalar.copy(out=finsb[:], in_=finP[:])
            nc.sync.dma_start(out=res_p[0:5], in_=finsb[:])

    nc.compile()
    return nc


def _consts():
    pri = np.broadcast_to((P - np.arange(P, dtype=np.float32))[None, :],
                          (NH, P)).copy()
    # r0c[h]: dram row base (+128) of heatmap h within its source tensor
    # row base within the concatenated [2,B_LOC,L,128,512] data tensor
    r0c = np.empty((NH, 1), dtype=np.float32)
    for h in range(NH):
        r0c[h, 0] = (h // HH) * HH * P + (h % HH) * P + P
    ones = np.ones((P, 1), dtype=np.float32)
    ident = np.eye(P, dtype=np.float32)
    return {"pri": pri, "r0c": r0c, "onesv": ones, "ident": ident}


def kernel(output: np.ndarray, target: np.ndarray) -> np.ndarray:
    global LAST_RESULTS
    from concourse.bass_utils import run_bass_kernel_spmd

    if "nc" not in _CACHE:
        _CACHE["nc"] = _build()
    nc = _CACHE["nc"]

    output = np.ascontiguousarray(output, dtype=np.float32)
    target = np.ascontiguousarray(target, dtype=np.float32)
    consts = _consts()
    in_maps = []
    for c in range(NCORES):
        m = {"data": np.stack([output[c * B_LOC:(c + 1) * B_LOC],
                               target[c * B_LOC:(c + 1) * B_LOC]])}
        m.update(consts)
        in_maps.append(m)

    trace = os.environ.get("KERNEL_TRACE") == "1"
    res = run_bass_kernel_spmd(nc, in_maps, list(range(NCORES)), trace=trace)
    LAST_RESULTS = res

    l2_sum = 0.0
    ang_sum = 0.0
    dist_sum = 0.0
    for c in range(NCORES):
        r = np.asarray(res.results[c]["res"], dtype=np.float64).reshape(-1)
        l2_sum += r[0]
        ang_sum += (r[1] + r[2]) / (L * L)
        dist_sum += (r[3] + r[4]) / (L * L)

    l2 = l2_sum / (B * L * H * W)
    w = 1.0 + ang_sum + np.log(dist_sum + 1e-10)
    loss = l2 * w
    return np.array([loss, l2, w, ang_sum, dist_sum], dtype=np.float32)

